# revision 31
# baseline (speedup 1.0000x reference)
"""Self-contained Trainium2 Bass kernel for a 12-head attention layer.

Problem: x[4,2048,768] -> attention(QKV projections, softmax, context),
NUM_HEADS=12, SIZE_PER_HEAD=64, additive mask from mask[4,2048].

Sharding over 8 NeuronCores: core c handles batch b=c//2 and head-group
hg=c%2 (6 heads, 384 feature columns).  Everything is local per core:
no collectives.

v3 design (ACT-bound): the per-core exp work (6 heads x 2048^2 = 25.2M
elements at 1 elem/lane/cycle @1.2GHz + 352c/inst) floors the Scalar
engine at ~220us with N=1024 ACTs, so the TensorEngine stream is
organized to fit just under that roof:

  per head-pair p, f-chunk g (512 wide), t-tile ti:
    scores  S^T[t,f]: head A -> psS[:,0:512], head B -> psS[:,512:1024]
            (one [128,1024] psum tile, two banks, double-buffered)
    exp     ONE ACT N=1024 over both heads' chunks:
            Exp(psS + adder[ti]) -> bf16 [128,1024]  (mask = ACT bias)
    ctx     per head: [65,512] psum accumulated over ti; the V tiles
            carry a 65th ones-column so row 64 is the softmax
            denominator (no separate denominator matmuls)
  normalization fully off the PE: DVE drain, DMA gather of denom rows,
  DVE reciprocal, gpsimd partition_broadcast, DVE multiply, DMA out.

Per-cycle budget: ACT 1147ns vs PE 2x scores + 2x ctx ~ 950ns + shared
projection work ~ 290ns -> both engines ~saturated, ACT binding.

Output per core: ctx^T [384,2048] f32; host transposes/concats.
"""

import numpy as np
import ml_dtypes

B, S, D = 4, 2048, 768
H, DH = 12, 64
HL = 6          # heads per core
DL = HL * DH    # 384 feature columns per core
NCORES = 8
P = 128
KO = 6          # full k-subtiles of the 768 contraction
NT = S // P     # 16 T-tiles
NG = 4          # f-chunks of 512 per head

_CACHE = {}


def _build(with_bias=False, ncores=NCORES):
    import concourse.mybir as mybir
    import concourse.tile as tile
    from concourse import bacc

    dt = mybir.dt
    Exp = mybir.ActivationFunctionType.Exp
    Alu = mybir.AluOpType

    nc = bacc.Bacc("TRN2", target_bir_lowering=False, debug=False,
                   num_devices=ncores)

    DE = D + 1 if with_bias else D
    WVC = HL * (DH + 1) if with_bias else DL   # 390 vs 384
    xT = nc.dram_tensor("xT", [DE, S], dt.bfloat16, kind="ExternalInput")
    wq = nc.dram_tensor("wq", [DE, DL], dt.bfloat16, kind="ExternalInput")
    wk = nc.dram_tensor("wk", [DE, DL], dt.bfloat16, kind="ExternalInput")
    wv = nc.dram_tensor("wv", [DE, WVC], dt.bfloat16, kind="ExternalInput")
    adder = nc.dram_tensor("adder", [P, NT], dt.float32, kind="ExternalInput")
    out = nc.dram_tensor("out", [DL, S], dt.float32, kind="ExternalOutput")

    KE = KO + 1 if with_bias else KO

    with tile.TileContext(nc) as tc:
        with (
            tc.tile_pool(name="persist", bufs=1) as sb,
            tc.tile_pool(name="work", bufs=3) as work,
            tc.tile_pool(name="fin", bufs=2) as fin,
            tc.tile_pool(name="ps_s", bufs=2, space="PSUM") as ps_s,
            tc.tile_pool(name="ps_c", bufs=1, space="PSUM") as ps_c,
        ):
            # ---- input DMA ----
            xTs = sb.tile([P, KE, S], dt.bfloat16, tag="xTs")
            for ko in range(KO):
                nc.sync.dma_start(
                    xTs[:, ko, :], xT.ap()[ko * P:(ko + 1) * P, :])
            if with_bias:
                nc.sync.dma_start(xTs[0:1, KO, :], xT.ap()[D:D + 1, :])

            wqs = sb.tile([P, KE, DL], dt.bfloat16, tag="wqs")
            wks = sb.tile([P, KE, DL], dt.bfloat16, tag="wks")
            wvs = sb.tile([P, KE, WVC], dt.bfloat16, tag="wvs")
            adder_sb = sb.tile([P, NT], dt.float32, tag="adder")

            def dma_w(w_dram, w_sb, cols):
                # host already stores rows as [p, ko, m] (partition-major),
                # so this is a contiguous transfer, not a gather
                nc.sync.dma_start(
                    w_sb[:, 0:KO, 0:cols],
                    w_dram.ap()[0:D, :].rearrange("(p ko) m -> p ko m", p=P))
                if with_bias:
                    nc.sync.dma_start(w_sb[0:1, KO, 0:cols],
                                      w_dram.ap()[D:D + 1, :])

            dma_w(wq, wqs, DL)
            dma_w(wk, wks, DL)
            dma_w(wv, wvs, WVC)
            nc.sync.dma_start(adder_sb[:], adder.ap())

            # persistent projection outputs
            qt = sb.tile([P, 3, S], dt.bfloat16, tag="qt")   # Q^T/8 (+bias)
            kt = sb.tile([P, 3, S], dt.bfloat16, tag="kt")   # K^T (+bias)
            # V' token-major, 65-col head blocks (65th col = ones -> denom)
            # ones column FIRST in each head block: the ctx psum row 0 is
            # then the softmax denominator at partition 0, so the epilogue
            # reciprocal/broadcast needs no partition-moving DMAs
            vp = sb.tile([P, NT, HL, DH + 1], dt.bfloat16, tag="vp")
            if not with_bias:
                nc.gpsimd.memset(vp[:, :, :, 0:1], 1.0)

            # ---- projections ----
            def proj_qk(w_sb, dst, m, ns=(0, 1, 2, 3)):
                for n in ns:
                    pt = ps_c.tile([P, 512], dt.float32, tag="proj",
                                   name="pt", bufs=2)
                    for k in range(KE):
                        lhsT = (w_sb[:, k, m * P:(m + 1) * P] if k < KO
                                else w_sb[0:1, k, m * P:(m + 1) * P])
                        rhs = (xTs[:, k, n * 512:(n + 1) * 512] if k < KO
                               else xTs[0:1, k, n * 512:(n + 1) * 512])
                        nc.tensor.matmul(pt[:], lhsT, rhs,
                                         start=(k == 0), stop=(k == KE - 1))
                    nc.vector.tensor_copy(dst[:, m, n * 512:(n + 1) * 512],
                                          pt[:])

            def proj_v(mt):
                pt = ps_c.tile([P, 512], dt.float32, tag="proj", name="pt",
                               bufs=2)
                for k in range(KE):
                    lhsT = (xTs[:, k, mt * P:(mt + 1) * P] if k < KO
                            else xTs[0:1, k, mt * P:(mt + 1) * P])
                    rhs = wvs[:, k, 0:WVC] if k < KO else wvs[0:1, k, 0:WVC]
                    nc.tensor.matmul(pt[:, :WVC], lhsT, rhs,
                                     start=(k == 0), stop=(k == KE - 1))
                if with_bias:
                    nc.vector.tensor_copy(
                        vp[:, mt, :, :],
                        pt[:, :WVC].rearrange("p (h c) -> p h c", h=HL))
                else:
                    nc.vector.tensor_copy(
                        vp[:, mt, :, 1:DH + 1],
                        pt[:, :DL].rearrange("p (h c) -> p h c", h=HL))

            def epilogue(p, g, ctx_ps, fine=False):
                # ctx_ps: [hip] -> [DH+1, 512] psum; row 0 = denominator,
                # rows 1..64 = ctx.  All in-lane: copy, reciprocal of row
                # 0, partition_broadcast (rows 0..64), multiply on rows
                # 1..64, output DMA.  fine=True splits into 256-wide
                # chains so the kernel tail pipelines.
                nch = 2 if fine else 1
                cw = 512 // nch
                for hip in range(2):
                    cst = fin.tile([DH + 1, 512], dt.float32,
                                   tag=f"cst{hip}", name="cst", bufs=2)
                    nc.vector.tensor_copy(cst[:], ctx_ps[hip][:])
                    for ch in range(nch):
                        rr = fin.tile([1, cw], dt.float32, tag=f"rr{nch}",
                                      name="rr", bufs=2 * nch)
                        nc.vector.reciprocal(
                            rr[:], cst[0:1, ch * cw:(ch + 1) * cw])
                        bb = fin.tile([DH + 1, cw], dt.float32,
                                      tag=f"bb{nch}", name="bb",
                                      bufs=2 * nch)
                        nc.gpsimd.partition_broadcast(bb[:], rr[:])
                        otc = fin.tile([DH + 1, cw], dt.float32,
                                       tag=f"otc{nch}", bufs=3 * nch,
                                       name="otc")
                        # row 0 computes denom*recip (ignored); engine
                        # partition slices must be 32-aligned, DMAs not
                        nc.vector.tensor_tensor(
                            otc[:, :],
                            cst[:, ch * cw:(ch + 1) * cw],
                            bb[:, :], Alu.mult)
                        nc.sync.dma_start(
                            out.ap()[p * P + hip * DH:
                                     p * P + (hip + 1) * DH,
                                     g * 512 + ch * cw:
                                     g * 512 + (ch + 1) * cw],
                            otc[1:DH + 1, :])

            def attn_all(bg, pre_step):
                # One flattened software pipeline over all 192
                # (pair, g, ti) steps: scores+exp for step j, ctx for step
                # j-16.  The segment-sized lag means ctx deps are a full
                # exp-backlog old (the PE FIFO never waits on the ACT),
                # segment s's ctx drains exactly during segment s+1's
                # scores (ctx psum tiles stay single-buffered), and every
                # segment/pair boundary bubble disappears.  pre_step maps
                # step -> emission block run before that step (V-proj
                # blocks that hide under the exp backlog); bg interleaves
                # one background-projection instruction per step.
                LAG = 8
                ctx_tiles = {}
                exp_tiles = {}

                def seg_of(step):
                    p, r = divmod(step, 64)
                    return p, r // 16, r % 16

                for step in range(192 + LAG):
                    if step in pre_step:
                        pre_step[step]()
                    if step < 192:
                        p, g, ti = seg_of(step)
                        psS = ps_s.tile([P, 1024], dt.float32, tag="s",
                                        name="psS")
                        nc.tensor.matmul(
                            psS[:, 0:512],
                            kt[0:DH, p, ti * P:(ti + 1) * P],
                            qt[0:DH, p, g * 512:(g + 1) * 512],
                            start=True, stop=True)
                        nc.tensor.matmul(
                            psS[:, 512:1024],
                            kt[DH:P, p, ti * P:(ti + 1) * P],
                            qt[DH:P, p, g * 512:(g + 1) * 512],
                            start=True, stop=True)
                        # stage scores in SBUF (bf16, DVE 2x rate) so the
                        # ACT can run many steps behind the PE: the psum
                        # slot frees at DVE pace, letting scores sprint
                        # ahead while V/proj bursts occupy the PE
                        sst = work.tile([P, 1024], dt.bfloat16,
                                        tag="sst", name="sst", bufs=8)
                        nc.vector.tensor_copy(sst[:], psS[:])
                        et = work.tile([P, 1024], dt.bfloat16, tag="exp",
                                       name="et", bufs=LAG + 2)
                        nc.scalar.activation(et[:], sst[:], Exp,
                                             bias=adder_sb[:, ti:ti + 1],
                                             scale=1.0)
                        exp_tiles[step] = et
                    j = step - LAG
                    if j >= 0:
                        p, g, ti = seg_of(j)
                        seg = 4 * p + g
                        if ti == 0:
                            ctx_tiles[seg] = [
                                ps_c.tile([DH + 1, 512], dt.float32,
                                          tag=f"c{hip}", name="ctx_ps")
                                for hip in range(2)
                            ]
                        et = exp_tiles.pop(j)
                        for hip in range(2):
                            nc.tensor.matmul(
                                ctx_tiles[seg][hip][:],
                                vp[:, ti, 2 * p + hip, :],
                                et[:, hip * 512:(hip + 1) * 512],
                                start=(ti == 0), stop=(ti == NT - 1))
                        if ti == NT - 1:
                            epilogue(p, g, ctx_tiles.pop(seg),
                                     fine=(seg == 11))
                    pops = 2 if step < 40 else 1
                    for _ in range(pops):
                        if bg:
                            bg.pop(0)()

            # PE warm-up: garbage matmuls with no input deps run during the
            # initial DMA wait, releasing the HAM clock throttle.
            warm = sb.tile([P, 512], dt.bfloat16, tag="warm")
            nc.gpsimd.memset(warm[:], 0.0)
            wexp = sb.tile([P, 1], dt.bfloat16, tag="wexp")
            nc.scalar.activation(wexp[:], warm[:, 0:1], Exp)
            wpt = ps_s.tile([P, 1024], dt.float32, tag="s", name="wpt")
            for wi in range(26):
                nc.tensor.matmul(wpt[:, 0:512], warm[:, 0:P], warm[:],
                                 start=(wi == 0), stop=(wi == 25))

            def proj_thunks_qk(w_sb, dst, m, n):
                # one-instruction-per-thunk version of proj_qk(m, (n,))
                state = {}

                def mk(k):
                    def t():
                        if k == 0:
                            state["pt"] = ps_c.tile([P, 512], dt.float32,
                                                    tag="proj", name="pt",
                                                    bufs=2)
                        lhsT = (w_sb[:, k, m * P:(m + 1) * P] if k < KO
                                else w_sb[0:1, k, m * P:(m + 1) * P])
                        rhs = (xTs[:, k, n * 512:(n + 1) * 512] if k < KO
                               else xTs[0:1, k, n * 512:(n + 1) * 512])
                        nc.tensor.matmul(state["pt"][:], lhsT, rhs,
                                         start=(k == 0), stop=(k == KE - 1))
                    return t

                def cp():
                    nc.vector.tensor_copy(
                        dst[:, m, n * 512:(n + 1) * 512], state["pt"][:])

                return [mk(k) for k in range(KE)] + [cp]

            # prefix: only what scores(0..3) need up front; remaining k-m0
            # chunks drip in just ahead of their ti via the bg queue (2
            # pops/step early on); the V projections run AFTER g0's first
            # scores, hidden under the ACT's exp backlog.
            proj_qk(wqs, qt, 0, ns=(0,))
            proj_qk(wks, kt, 0, ns=(0,))

            def v_block(lo, hi):
                def f():
                    for mt in range(lo, hi):
                        proj_v(mt)
                return f

            bg = []
            for n in (1, 2, 3):
                bg += proj_thunks_qk(wks, kt, 0, n)
            for n in (1, 2, 3):
                bg += proj_thunks_qk(wqs, qt, 0, n)
            for n in range(4):
                bg += proj_thunks_qk(wks, kt, 1, n)
            for n in range(4):
                bg += proj_thunks_qk(wqs, qt, 1, n)
            for n in range(4):
                bg += proj_thunks_qk(wks, kt, 2, n)
            for n in range(4):
                bg += proj_thunks_qk(wqs, qt, 2, n)
            bg += [lambda: None] * (300 - len(bg))
            attn_all(bg, {8: v_block(0, 8), 16: v_block(8, NT)})

    nc.compile()
    return nc


def _prep_core_inputs(c, x, Wq, bq, Wk, bk, Wv, bv, mask, with_bias):
    bf16 = ml_dtypes.bfloat16
    b, hg = c // 2, c % 2
    cols = slice(hg * DL, (hg + 1) * DL)
    DE = D + 1 if with_bias else D

    xT_aug = np.empty((DE, S), dtype=bf16)
    xT_aug[:D] = x[b].T.astype(bf16)
    if with_bias:
        xT_aug[D] = np.float32(1.0)

    def pko(w):
        # store weight rows partition-major ([p, ko] instead of [ko, p])
        # so the on-chip DMA into [P, KO, cols] is contiguous
        c = w.shape[1]
        return w.reshape(KO, P, c).transpose(1, 0, 2).reshape(D, c)

    wq_aug = np.empty((DE, DL), dtype=bf16)
    wq_aug[:D] = pko((Wq[:, cols] / 8.0).astype(bf16))
    wk_aug = np.empty((DE, DL), dtype=bf16)
    wk_aug[:D] = pko(Wk[:, cols].astype(bf16))
    if with_bias:
        wq_aug[D] = (bq[cols] / 8.0).astype(bf16)
        wk_aug[D] = bk[cols].astype(bf16)
        wv_aug = np.zeros((DE, HL * (DH + 1)), dtype=bf16)
        wv_loc = Wv[:, cols].astype(np.float32)
        bv_loc = bv[cols].astype(np.float32)
        for j in range(HL):
            wv_aug[:D, j * (DH + 1) + 1:(j + 1) * (DH + 1)] = \
                wv_loc[:, j * DH:(j + 1) * DH].astype(bf16)
            wv_aug[D, j * (DH + 1) + 1:(j + 1) * (DH + 1)] = \
                bv_loc[j * DH:(j + 1) * DH].astype(bf16)
            wv_aug[D, j * (DH + 1)] = np.float32(1.0)
        wv_aug[:D] = pko(wv_aug[:D].copy())
    else:
        wv_aug = np.empty((DE, DL), dtype=bf16)
        wv_aug[:D] = pko(Wv[:, cols].astype(bf16))

    add = ((mask[b].astype(np.float32) - 1.0) * 10000.0)
    adder_t = add.reshape(NT, P).T.copy()   # [128,16]: [p, ti]

    return {"xT": xT_aug, "wq": wq_aug, "wk": wk_aug, "wv": wv_aug,
            "adder": np.ascontiguousarray(adder_t, dtype=np.float32)}


def kernel(x, Wq, bq, Wk, bk, Wv, bv, mask, _trace=False):
    from concourse.bass_utils import run_bass_kernel_spmd

    x = np.asarray(x, dtype=np.float32)
    Wq = np.asarray(Wq, dtype=np.float32)
    bq = np.asarray(bq, dtype=np.float32)
    Wk = np.asarray(Wk, dtype=np.float32)
    bk = np.asarray(bk, dtype=np.float32)
    Wv = np.asarray(Wv, dtype=np.float32)
    bv = np.asarray(bv, dtype=np.float32)
    mask = np.asarray(mask)

    with_bias = bool(bq.any() or bk.any() or bv.any())
    key = ("nc", with_bias)
    if key not in _CACHE:
        _CACHE[key] = _build(with_bias=with_bias)
    nc = _CACHE[key]

    in_maps = [_prep_core_inputs(c, x, Wq, bq, Wk, bk, Wv, bv, mask,
                                 with_bias)
               for c in range(NCORES)]
    res = run_bass_kernel_spmd(nc, in_maps, core_ids=list(range(NCORES)),
                               trace=_trace)
    if _trace:
        _CACHE["last_result"] = res

    full = np.empty((B, S, D), dtype=np.float32)
    for c in range(NCORES):
        b, hg = c // 2, c % 2
        full[b, :, hg * DL:(hg + 1) * DL] = res.results[c]["out"].T
    return full


# revision 32
# speedup vs baseline: 1.5395x; 1.5395x over previous
"""Self-contained Trainium2 Bass kernel for a 12-head attention layer.

Problem: x[4,2048,768] -> attention(QKV projections, softmax, context),
NUM_HEADS=12, SIZE_PER_HEAD=64, additive mask from mask[4,2048].

Sharding over 8 NeuronCores: core c handles batch b=c//2 and head-group
hg=c%2 (6 heads, 384 feature columns).  Everything is local per core:
no collectives.

v3 design (ACT-bound): the per-core exp work (6 heads x 2048^2 = 25.2M
elements at 1 elem/lane/cycle @1.2GHz + 352c/inst) floors the Scalar
engine at ~220us with N=1024 ACTs, so the TensorEngine stream is
organized to fit just under that roof:

  per head-pair p, f-chunk g (512 wide), t-tile ti:
    scores  S^T[t,f]: head A -> psS[:,0:512], head B -> psS[:,512:1024]
            (one [128,1024] psum tile, two banks, double-buffered)
    exp     ONE ACT N=1024 over both heads' chunks:
            Exp(psS + adder[ti]) -> bf16 [128,1024]  (mask = ACT bias)
    ctx     per head: [65,512] psum accumulated over ti; the V tiles
            carry a 65th ones-column so row 64 is the softmax
            denominator (no separate denominator matmuls)
  normalization fully off the PE: DVE drain, DMA gather of denom rows,
  DVE reciprocal, gpsimd partition_broadcast, DVE multiply, DMA out.

Per-cycle budget: ACT 1147ns vs PE 2x scores + 2x ctx ~ 950ns + shared
projection work ~ 290ns -> both engines ~saturated, ACT binding.

Output per core: ctx^T [384,2048] f32; host transposes/concats.
"""

import numpy as np
import ml_dtypes

B, S, D = 4, 2048, 768
H, DH = 12, 64
HL = 6          # heads per core
DL = HL * DH    # 384 feature columns per core
NCORES = 8
P = 128
KO = 6          # full k-subtiles of the 768 contraction
NT = S // P     # 16 T-tiles
NG = 4          # f-chunks of 512 per head

_CACHE = {}


def _build(with_bias=False, ncores=NCORES):
    import concourse.mybir as mybir
    import concourse.tile as tile
    from concourse import bacc

    dt = mybir.dt
    Exp = mybir.ActivationFunctionType.Exp
    Alu = mybir.AluOpType

    nc = bacc.Bacc("TRN2", target_bir_lowering=False, debug=False,
                   num_devices=ncores)

    DE = D + 1 if with_bias else D
    WVC = HL * (DH + 1) if with_bias else DL   # 390 vs 384
    xT = nc.dram_tensor("xT", [DE, S], dt.bfloat16, kind="ExternalInput")
    wq = nc.dram_tensor("wq", [DE, DL], dt.bfloat16, kind="ExternalInput")
    wk = nc.dram_tensor("wk", [DE, DL], dt.bfloat16, kind="ExternalInput")
    wv = nc.dram_tensor("wv", [DE, WVC], dt.bfloat16, kind="ExternalInput")
    adder = nc.dram_tensor("adder", [P, NT], dt.float32, kind="ExternalInput")
    out = nc.dram_tensor("out", [DL, S], dt.float32, kind="ExternalOutput")

    KE = KO + 1 if with_bias else KO

    with tile.TileContext(nc) as tc:
        with (
            tc.tile_pool(name="persist", bufs=1) as sb,
            tc.tile_pool(name="work", bufs=3) as work,
            tc.tile_pool(name="fin", bufs=2) as fin,
            tc.tile_pool(name="ps_s", bufs=2, space="PSUM") as ps_s,
            tc.tile_pool(name="ps_c", bufs=1, space="PSUM") as ps_c,
        ):
            # ---- input DMA ----
            xTs = sb.tile([P, KE, S], dt.bfloat16, tag="xTs")
            for ko in range(KO):
                nc.sync.dma_start(
                    xTs[:, ko, :], xT.ap()[ko * P:(ko + 1) * P, :])
            if with_bias:
                nc.sync.dma_start(xTs[0:1, KO, :], xT.ap()[D:D + 1, :])

            wqs = sb.tile([P, KE, DL], dt.bfloat16, tag="wqs")
            wks = sb.tile([P, KE, DL], dt.bfloat16, tag="wks")
            wvs = sb.tile([P, KE, WVC], dt.bfloat16, tag="wvs")
            adder_sb = sb.tile([P, NT], dt.float32, tag="adder")

            def dma_w(w_dram, w_sb, cols):
                # host already stores rows as [p, ko, m] (partition-major),
                # so this is a contiguous transfer, not a gather
                nc.sync.dma_start(
                    w_sb[:, 0:KO, 0:cols],
                    w_dram.ap()[0:D, :].rearrange("(p ko) m -> p ko m", p=P))
                if with_bias:
                    nc.sync.dma_start(w_sb[0:1, KO, 0:cols],
                                      w_dram.ap()[D:D + 1, :])

            dma_w(wq, wqs, DL)
            dma_w(wk, wks, DL)
            dma_w(wv, wvs, WVC)
            nc.sync.dma_start(adder_sb[:], adder.ap())

            # persistent projection outputs
            qt = sb.tile([P, 3, S], dt.bfloat16, tag="qt")   # Q^T/8 (+bias)
            kt = sb.tile([P, 3, S], dt.bfloat16, tag="kt")   # K^T (+bias)
            # V' token-major, 65-col head blocks (65th col = ones -> denom)
            # ones column FIRST in each head block: the ctx psum row 0 is
            # then the softmax denominator at partition 0, so the epilogue
            # reciprocal/broadcast needs no partition-moving DMAs
            vp = sb.tile([P, NT, HL, DH + 1], dt.bfloat16, tag="vp")
            if not with_bias:
                nc.gpsimd.memset(vp[:, :, :, 0:1], 1.0)

            # ---- projections ----
            def proj_qk(w_sb, dst, m, ns=(0, 1, 2, 3)):
                for n in ns:
                    pt = ps_c.tile([P, 512], dt.float32, tag="proj",
                                   name="pt", bufs=2)
                    for k in range(KE):
                        lhsT = (w_sb[:, k, m * P:(m + 1) * P] if k < KO
                                else w_sb[0:1, k, m * P:(m + 1) * P])
                        rhs = (xTs[:, k, n * 512:(n + 1) * 512] if k < KO
                               else xTs[0:1, k, n * 512:(n + 1) * 512])
                        nc.tensor.matmul(pt[:], lhsT, rhs,
                                         start=(k == 0), stop=(k == KE - 1))
                    nc.vector.tensor_copy(dst[:, m, n * 512:(n + 1) * 512],
                                          pt[:])

            def proj_v(mt):
                pt = ps_c.tile([P, 512], dt.float32, tag="proj", name="pt",
                               bufs=2)
                for k in range(KE):
                    lhsT = (xTs[:, k, mt * P:(mt + 1) * P] if k < KO
                            else xTs[0:1, k, mt * P:(mt + 1) * P])
                    rhs = wvs[:, k, 0:WVC] if k < KO else wvs[0:1, k, 0:WVC]
                    nc.tensor.matmul(pt[:, :WVC], lhsT, rhs,
                                     start=(k == 0), stop=(k == KE - 1))
                if with_bias:
                    nc.vector.tensor_copy(
                        vp[:, mt, :, :],
                        pt[:, :WVC].rearrange("p (h c) -> p h c", h=HL))
                else:
                    nc.vector.tensor_copy(
                        vp[:, mt, :, 1:DH + 1],
                        pt[:, :DL].rearrange("p (h c) -> p h c", h=HL))

            def epilogue(p, g, ctx_ps, fine=False):
                # ctx_ps: [hip] -> [DH+1, 512] psum; row 0 = denominator,
                # rows 1..64 = ctx.  Drain psum fast (DVE), then gather the
                # denom rows to [128,n] via sbuf-sbuf DMA so the reciprocal
                # uses all DVE lanes, scatter back, gpsimd-broadcast,
                # multiply, DMA out.  fine=True splits into 256-wide chains
                # so the kernel tail pipelines.
                csts = []
                for hip in range(2):
                    cst = fin.tile([DH + 1, 512], dt.float32,
                                   tag=f"cst{hip}", name="cst", bufs=2)
                    nc.vector.tensor_copy(cst[:], ctx_ps[hip][:])
                    csts.append(cst)
                nch = 2 if fine else 1
                cw = 512 // nch
                for ch in range(nch):
                    dcol = fin.tile([P, 8 // nch], dt.float32,
                                    tag=f"dcol{nch}", name="dcol",
                                    bufs=2 * nch)
                    for hip in range(2):
                        nc.sync.dma_start(
                            dcol[:, hip * 4 // nch:(hip + 1) * 4 // nch],
                            csts[hip][0:1, ch * cw:(ch + 1) * cw])
                    rc = fin.tile([P, 8 // nch], dt.float32,
                                  tag=f"rc{nch}", name="rc", bufs=2 * nch)
                    nc.vector.reciprocal(rc[:], dcol[:])
                    for hip in range(2):
                        rrow = fin.tile([1, cw], dt.float32,
                                        tag=f"rrow{nch}", name="rrow",
                                        bufs=2 * nch)
                        nc.sync.dma_start(
                            rrow[:],
                            rc[:, hip * 4 // nch:(hip + 1) * 4 // nch])
                        bb = fin.tile([DH + 1, cw], dt.float32,
                                      tag=f"bb{nch}", name="bb",
                                      bufs=2 * nch)
                        nc.gpsimd.partition_broadcast(bb[:], rrow[:])
                        otc = fin.tile([DH + 1, cw], dt.float32,
                                       tag=f"otc{nch}", bufs=3 * nch,
                                       name="otc")
                        nc.vector.tensor_tensor(
                            otc[:, :],
                            csts[hip][:, ch * cw:(ch + 1) * cw],
                            bb[:, :], Alu.mult)
                        nc.sync.dma_start(
                            out.ap()[p * P + hip * DH:
                                     p * P + (hip + 1) * DH,
                                     g * 512 + ch * cw:
                                     g * 512 + (ch + 1) * cw],
                            otc[1:DH + 1, :])

            def attn_all(bg, pre_step):
                # One flattened software pipeline over all 192
                # (pair, g, ti) steps: scores+exp for step j, ctx for step
                # j-16.  The segment-sized lag means ctx deps are a full
                # exp-backlog old (the PE FIFO never waits on the ACT),
                # segment s's ctx drains exactly during segment s+1's
                # scores (ctx psum tiles stay single-buffered), and every
                # segment/pair boundary bubble disappears.  pre_step maps
                # step -> emission block run before that step (V-proj
                # blocks that hide under the exp backlog); bg interleaves
                # one background-projection instruction per step.
                LAG = 8
                ctx_tiles = {}
                exp_tiles = {}

                def seg_of(step):
                    p, r = divmod(step, 64)
                    return p, r // 16, r % 16

                for step in range(192 + LAG):
                    if step in pre_step:
                        pre_step[step]()
                    if step < 192:
                        p, g, ti = seg_of(step)
                        psS = ps_s.tile([P, 1024], dt.float32, tag="s",
                                        name="psS")
                        nc.tensor.matmul(
                            psS[:, 0:512],
                            kt[0:DH, p, ti * P:(ti + 1) * P],
                            qt[0:DH, p, g * 512:(g + 1) * 512],
                            start=True, stop=True)
                        nc.tensor.matmul(
                            psS[:, 512:1024],
                            kt[DH:P, p, ti * P:(ti + 1) * P],
                            qt[DH:P, p, g * 512:(g + 1) * 512],
                            start=True, stop=True)
                        et = work.tile([P, 1024], dt.bfloat16, tag="exp",
                                       name="et", bufs=LAG + 2)
                        nc.scalar.activation(et[:], psS[:], Exp,
                                             bias=adder_sb[:, ti:ti + 1],
                                             scale=1.0)
                        exp_tiles[step] = et
                    j = step - LAG
                    if j >= 0:
                        p, g, ti = seg_of(j)
                        seg = 4 * p + g
                        if ti == 0:
                            ctx_tiles[seg] = [
                                ps_c.tile([DH + 1, 512], dt.float32,
                                          tag=f"c{hip}", name="ctx_ps")
                                for hip in range(2)
                            ]
                        et = exp_tiles.pop(j)
                        for hip in range(2):
                            nc.tensor.matmul(
                                ctx_tiles[seg][hip][:],
                                vp[:, ti, 2 * p + hip, :],
                                et[:, hip * 512:(hip + 1) * 512],
                                start=(ti == 0), stop=(ti == NT - 1))
                        if ti == NT - 1:
                            epilogue(p, g, ctx_tiles.pop(seg),
                                     fine=(seg == 11))
                    pops = 2 if step < 40 else 1
                    for _ in range(pops):
                        if bg:
                            bg.pop(0)()

            # PE warm-up: garbage matmuls with no input deps run during the
            # initial DMA wait, releasing the HAM clock throttle.
            warm = sb.tile([P, 512], dt.bfloat16, tag="warm")
            nc.gpsimd.memset(warm[:], 0.0)
            wexp = sb.tile([P, 1], dt.bfloat16, tag="wexp")
            nc.scalar.activation(wexp[:], warm[:, 0:1], Exp)
            wpt = ps_s.tile([P, 1024], dt.float32, tag="s", name="wpt")
            for wi in range(42):
                nc.tensor.matmul(wpt[:, 0:512], warm[:, 0:P], warm[:],
                                 start=(wi == 0), stop=(wi == 41))

            def proj_thunks_qk(w_sb, dst, m, n):
                # one-instruction-per-thunk version of proj_qk(m, (n,))
                state = {}

                def mk(k):
                    def t():
                        if k == 0:
                            state["pt"] = ps_c.tile([P, 512], dt.float32,
                                                    tag="proj", name="pt",
                                                    bufs=2)
                        lhsT = (w_sb[:, k, m * P:(m + 1) * P] if k < KO
                                else w_sb[0:1, k, m * P:(m + 1) * P])
                        rhs = (xTs[:, k, n * 512:(n + 1) * 512] if k < KO
                               else xTs[0:1, k, n * 512:(n + 1) * 512])
                        nc.tensor.matmul(state["pt"][:], lhsT, rhs,
                                         start=(k == 0), stop=(k == KE - 1))
                    return t

                def cp():
                    nc.vector.tensor_copy(
                        dst[:, m, n * 512:(n + 1) * 512], state["pt"][:])

                return [mk(k) for k in range(KE)] + [cp]

            # prefix: only what scores(0..3) need up front; remaining k-m0
            # chunks drip in just ahead of their ti via the bg queue (2
            # pops/step early on); the V projections run AFTER g0's first
            # scores, hidden under the ACT's exp backlog.
            proj_qk(wqs, qt, 0, ns=(0,))
            proj_qk(wks, kt, 0, ns=(0,))

            def v_block(lo, hi):
                def f():
                    for mt in range(lo, hi):
                        proj_v(mt)
                return f

            bg = []
            for n in (1, 2, 3):
                bg += proj_thunks_qk(wks, kt, 0, n)
            for n in (1, 2, 3):
                bg += proj_thunks_qk(wqs, qt, 0, n)
            for n in range(4):
                bg += proj_thunks_qk(wks, kt, 1, n)
            for n in range(4):
                bg += proj_thunks_qk(wqs, qt, 1, n)
            for n in range(4):
                bg += proj_thunks_qk(wks, kt, 2, n)
            for n in range(4):
                bg += proj_thunks_qk(wqs, qt, 2, n)
            bg += [lambda: None] * (300 - len(bg))
            attn_all(bg, {8: v_block(0, 8), 16: v_block(8, NT)})

    nc.compile()
    return nc


def _prep_core_inputs(c, x, Wq, bq, Wk, bk, Wv, bv, mask, with_bias):
    bf16 = ml_dtypes.bfloat16
    b, hg = c // 2, c % 2
    cols = slice(hg * DL, (hg + 1) * DL)
    DE = D + 1 if with_bias else D

    xT_aug = np.empty((DE, S), dtype=bf16)
    xT_aug[:D] = x[b].T.astype(bf16)
    if with_bias:
        xT_aug[D] = np.float32(1.0)

    def pko(w):
        # store weight rows partition-major ([p, ko] instead of [ko, p])
        # so the on-chip DMA into [P, KO, cols] is contiguous
        c = w.shape[1]
        return w.reshape(KO, P, c).transpose(1, 0, 2).reshape(D, c)

    wq_aug = np.empty((DE, DL), dtype=bf16)
    wq_aug[:D] = pko((Wq[:, cols] / 8.0).astype(bf16))
    wk_aug = np.empty((DE, DL), dtype=bf16)
    wk_aug[:D] = pko(Wk[:, cols].astype(bf16))
    if with_bias:
        wq_aug[D] = (bq[cols] / 8.0).astype(bf16)
        wk_aug[D] = bk[cols].astype(bf16)
        wv_aug = np.zeros((DE, HL * (DH + 1)), dtype=bf16)
        wv_loc = Wv[:, cols].astype(np.float32)
        bv_loc = bv[cols].astype(np.float32)
        for j in range(HL):
            wv_aug[:D, j * (DH + 1) + 1:(j + 1) * (DH + 1)] = \
                wv_loc[:, j * DH:(j + 1) * DH].astype(bf16)
            wv_aug[D, j * (DH + 1) + 1:(j + 1) * (DH + 1)] = \
                bv_loc[j * DH:(j + 1) * DH].astype(bf16)
            wv_aug[D, j * (DH + 1)] = np.float32(1.0)
        wv_aug[:D] = pko(wv_aug[:D].copy())
    else:
        wv_aug = np.empty((DE, DL), dtype=bf16)
        wv_aug[:D] = pko(Wv[:, cols].astype(bf16))

    add = ((mask[b].astype(np.float32) - 1.0) * 10000.0)
    adder_t = add.reshape(NT, P).T.copy()   # [128,16]: [p, ti]

    return {"xT": xT_aug, "wq": wq_aug, "wk": wk_aug, "wv": wv_aug,
            "adder": np.ascontiguousarray(adder_t, dtype=np.float32)}


def kernel(x, Wq, bq, Wk, bk, Wv, bv, mask, _trace=False):
    from concourse.bass_utils import run_bass_kernel_spmd

    x = np.asarray(x, dtype=np.float32)
    Wq = np.asarray(Wq, dtype=np.float32)
    bq = np.asarray(bq, dtype=np.float32)
    Wk = np.asarray(Wk, dtype=np.float32)
    bk = np.asarray(bk, dtype=np.float32)
    Wv = np.asarray(Wv, dtype=np.float32)
    bv = np.asarray(bv, dtype=np.float32)
    mask = np.asarray(mask)

    with_bias = bool(bq.any() or bk.any() or bv.any())
    key = ("nc", with_bias)
    if key not in _CACHE:
        _CACHE[key] = _build(with_bias=with_bias)
    nc = _CACHE[key]

    in_maps = [_prep_core_inputs(c, x, Wq, bq, Wk, bk, Wv, bv, mask,
                                 with_bias)
               for c in range(NCORES)]
    res = run_bass_kernel_spmd(nc, in_maps, core_ids=list(range(NCORES)),
                               trace=_trace)
    if _trace:
        _CACHE["last_result"] = res

    full = np.empty((B, S, D), dtype=np.float32)
    for c in range(NCORES):
        b, hg = c // 2, c % 2
        full[b, :, hg * DL:(hg + 1) * DL] = res.results[c]["out"].T
    return full
